# revision 1
# baseline (speedup 1.0000x reference)
"""Trainium2 Bass kernel: BiologicalAttention (mask-modulated multi-head attention).

Full computation:
    qkv = x @ W_qkv + b_qkv                         [B, N, 3, H, D]
    S   = (q @ k^T) * D**-0.5 * (0.1 + 0.9*mask)    [B, H, N, N]
    P   = softmax(S, axis=-1)
    out = (P @ v) reshaped to [B, N, C]
    y   = out @ W_out + b_out

Sharding (8 cores): core c handles batch b = c//2 and a 4-head group
g = c%2 (heads 4g..4g+3).  Each core computes a partial y for its batch
(its heads' contribution to the output projection); the host sums the
two partials per batch and adds b_out.

On-core layout (all data fp32; matmul operands in the hot loop are
materialized as float32r — same bytes, rounded — which streams through
the PE at 1 cycle/column instead of fp32's 4):
  - qT/kT stored transposed [4*32, N] with head h on partitions 32h..32h+31,
    so QK^T runs as 4 concurrent K=32 row-tiled matmuls (tile_position).
  - Scores are computed TRANSPOSED: T[m, n] = sum_d k[m,d] q[n,d], so the
    softmax denominator (sum over keys m = partitions) comes from a matmul:
    V is stored [m, d] with a ones-column appended, so P@[v|1] yields both
    the attention output (rows 0..31) and the softmax sums (row 32).
  - The pre-softmax mask multiply streams from PSUM through the DVE
    (tensor_mul, fused eviction) and exp runs on the scalar engine.
  - Normalization is deferred past P@V: O_h is scaled by 1/sums_h (per
    query n) while evicting PSUM, just before the output projection.
  - The n (query) axis is processed in 4 passes of 512; each pass gives
    every head its own PSUM accumulator bank, and each m-tile's mask chunk
    is streamed from HBM exactly once.
  - Host pre-folds scale into the mask: maskT = ((0.1+0.9*mask)*D^-0.5).T,
    and pre-augments weights with bias rows (x^T gets a ones row).
"""

import numpy as np
from contextlib import ExitStack

import concourse.bass as bass
import concourse.tile as tile
import concourse.mybir as mybir
from concourse import bacc

f32 = mybir.dt.float32
f32r = mybir.dt.float32r
bf16 = mybir.dt.bfloat16
Act = mybir.ActivationFunctionType

# problem shape (hardcoded per contract)
B, N, C, H = 4, 2048, 256, 8
D = 32
SCALE = D ** -0.5
HPC = 4                # heads per core
HD = HPC * D           # 128
VW = HPC * (D + 1)     # 132: per-m-tile v-store width ([v_h | 1] x 4 heads)
NCORES = 8


def build_program(n=N, debug=False, reps=1, gp_frac=0):
    """Build the SPMD Bass program for one core's shard. Same program runs
    on all 8 cores with different input bindings.

    reps: repeat the whole computation (timing aid: device time scales with
    reps while per-call dispatch overhead stays fixed).
    gp_frac: offload gp_frac/8 of the mask-multiply units to
    ACT-evict + GPSIMD-multiply (the DVE pays a pipe-drain tax ~2x its
    nominal throughput, so spreading the elementwise work pays off).
    """
    NQ = 4                 # n (query) passes
    CH = n // NQ           # 512 at full size
    MT = n // 128          # m-tiles (key tiles)
    TE = 2                 # m-tiles sharing one Exp op
    assert MT % TE == 0

    nc = bacc.Bacc("TRN2", target_bir_lowering=False, debug=debug)

    xT_d = nc.dram_tensor("xT", [C, n], f32, kind="ExternalInput")
    maskT_d = nc.dram_tensor("maskT", [n, n], f32, kind="ExternalInput")
    wq_d = nc.dram_tensor("wq", [C + 1, HD], f32, kind="ExternalInput")
    wk_d = nc.dram_tensor("wk", [C + 1, HD], f32, kind="ExternalInput")
    wv_d = nc.dram_tensor("wv", [C + 1, VW], f32, kind="ExternalInput")
    wo_d = nc.dram_tensor("wo", [HD, C], f32, kind="ExternalInput")
    y_d = nc.dram_tensor("y", [n, C], f32, kind="ExternalOutput")

    with tile.TileContext(nc) as tc, ExitStack() as ctx:
        const = ctx.enter_context(tc.tile_pool(name="const", bufs=1))
        maskp = ctx.enter_context(tc.tile_pool(name="maskp", bufs=6))
        tpool = ctx.enter_context(tc.tile_pool(name="tpool", bufs=3))
        ppool = ctx.enter_context(tc.tile_pool(name="ppool", bufs=3))
        ypool = ctx.enter_context(tc.tile_pool(name="ypool", bufs=2))
        spool = ctx.enter_context(tc.tile_pool(name="spool", bufs=2))
        rpool = ctx.enter_context(tc.tile_pool(name="rpool", bufs=1))
        psT = ctx.enter_context(tc.tile_pool(name="psT", bufs=2, space="PSUM"))
        psO = ctx.enter_context(tc.tile_pool(name="psO", bufs=4, space="PSUM"))

        # ---------------- constants / inputs ----------------
        xc0 = tpool.tile([128, n], f32, name="xc0", tag="T_tile")
        xc1 = tpool.tile([128, n], f32, name="xc1", tag="T_tile")
        nc.sync.dma_start(xc0[:], xT_d[0:128, :])
        nc.sync.dma_start(xc1[:], xT_d[128:256, :])
        ones_row = const.tile([1, n], f32, tag="ones_row")
        nc.vector.memset(ones_row[:], 1.0)
        zrow = const.tile([1, CH], bf16, tag="zrow")
        nc.vector.memset(zrow[:], 0.0)
        zc = const.tile([1, 128], bf16, tag="zc")
        nc.vector.memset(zc[:], 0.0)

        wq_sb = const.tile([128, 2 * HD], f32, tag="wq_sb")
        wk_sb = const.tile([128, 2 * HD], f32, tag="wk_sb")
        wv_sb = const.tile([128, 2 * VW], f32, tag="wv_sb")
        wqb = const.tile([1, HD], f32, tag="wqb")
        wkb = const.tile([1, HD], f32, tag="wkb")
        wvb = const.tile([1, VW], f32, tag="wvb")
        wo_f = const.tile([128, C], f32, tag="wo_f")
        wo_sb = const.tile([128, C], f32r, tag="wo_sb")
        for sb, d_, w in ((wq_sb, wq_d, HD), (wk_sb, wk_d, HD), (wv_sb, wv_d, VW)):
            nc.sync.dma_start(sb[:, 0:w], d_[0:128, :])
            nc.sync.dma_start(sb[:, w:2 * w], d_[128:256, :])
        nc.sync.dma_start(wqb[:], wq_d[256:257, :])
        nc.sync.dma_start(wkb[:], wk_d[256:257, :])
        nc.sync.dma_start(wvb[:], wv_d[256:257, :])
        nc.sync.dma_start(wo_f[:], wo_d[:])
        nc.scalar.copy(wo_sb[:], wo_f[:])

        qT_sb = const.tile([128, n], f32r, tag="qT_sb")
        kT_sb = const.tile([128, n], f32r, tag="kT_sb")
        v_store = const.tile([128, MT * VW], f32r, tag="v_store")
        O_allT = const.tile([128, n], f32r, tag="O_allT")

        # f32r copies of the phase-1 matmul operands (DVE is idle here)
        xr0 = const.tile([128, n], f32r, tag="xr0")
        xr1 = const.tile([128, n], f32r, tag="xr1")
        ones_r = const.tile([1, n], f32r, tag="ones_r")
        wq_r = const.tile([128, 2 * HD], f32r, tag="wq_r")
        wk_r = const.tile([128, 2 * HD], f32r, tag="wk_r")
        wv_r = const.tile([128, 2 * VW], f32r, tag="wv_r")
        wqb_r = const.tile([1, HD], f32r, tag="wqb_r")
        wkb_r = const.tile([1, HD], f32r, tag="wkb_r")
        wvb_r = const.tile([1, VW], f32r, tag="wvb_r")
        nc.vector.tensor_copy(xr0[:], xc0[:])
        nc.vector.tensor_copy(xr1[:], xc1[:])
        nc.vector.tensor_copy(ones_r[:], ones_row[:])
        nc.vector.tensor_copy(wq_r[:], wq_sb[:])
        nc.vector.tensor_copy(wk_r[:], wk_sb[:])
        nc.vector.tensor_copy(wv_r[:], wv_sb[:])
        nc.vector.tensor_copy(wqb_r[:], wqb[:])
        nc.vector.tensor_copy(wkb_r[:], wkb[:])
        nc.vector.tensor_copy(wvb_r[:], wvb[:])

        for _rep in range(reps):
            # ------------- phase 1: QKV projections (plain fp32) -------------
            # qT/kT: [32h+d, n] = W.T @ x.T (+ bias via ones-row rank-1 term);
            # evictions round to f32r for the hot-loop matmuls.
            for chunk in range(NQ):
                cs = bass.ts(chunk, CH)
                for dst, w_sb, w_b in ((qT_sb, wq_r, wqb_r), (kT_sb, wk_r, wkb_r)):
                    pq = psO.tile([128, CH], f32, name="pq", tag="psO")
                    nc.tensor.matmul(pq[:], lhsT=w_sb[:, 0:HD], rhs=xr0[:, cs],
                                     start=True, stop=False)
                    nc.tensor.matmul(pq[:], lhsT=w_sb[:, HD:2 * HD],
                                     rhs=xr1[:, cs], start=False, stop=False)
                    nc.tensor.matmul(pq[:], lhsT=w_b[0:1, :],
                                     rhs=ones_r[0:1, cs],
                                     start=False, stop=True)
                    nc.scalar.copy(dst[:, cs], pq[:])
            # v: [m, (v_h | 1) x 4] per m-tile; ones col comes from the bias row
            for t in range(MT):
                ms = bass.ts(t, 128)
                pv = psO.tile([128, VW], f32, name="pv", tag="psO")
                nc.tensor.matmul(pv[:], lhsT=xr0[:, ms], rhs=wv_r[:, 0:VW],
                                 start=True, stop=False)
                nc.tensor.matmul(pv[:], lhsT=xr1[:, ms],
                                 rhs=wv_r[:, VW:2 * VW],
                                 start=False, stop=False)
                nc.tensor.matmul(pv[:], lhsT=ones_r[0:1, ms], rhs=wvb_r[0:1, :],
                                 start=False, stop=True)
                nc.scalar.copy(v_store[:, t * VW:(t + 1) * VW], pv[:])

            # ------------- phase 2: attention, one pass per n-chunk ----------
            # The previous pass's epilogue is emitted after the first m-tile
            # pair of the next pass (engines execute in program order, so
            # this hides the serial sums->bcast->recip chain behind live
            # QK^T/mul work instead of stalling every engine at the
            # pass boundary).
            def epilogue_a(q, po):
                qs = bass.ts(q, CH)
                # sums (psum row 32 of each head) -> partition 32h (aligned)
                sraw = spool.tile([128, CH], f32, name="sraw", tag="sraw")
                for h in range(4):
                    nc.scalar.copy(sraw[32 * h:32 * h + 1, :],
                                   po[h][32:33, :])
                # broadcast each head's sums row across its 32 partitions (DMA)
                r_raw = rpool.tile([128, CH], f32, name="r_raw", tag="r_raw")
                src = sraw[:]
                bc = bass.AP(tensor=src.tensor, offset=src.offset,
                             ap=[[32 * src.ap[0][0], 4], [0, 32], src.ap[-1]])
                nc.sync.dma_start(r_raw[:], bc)
                r_scr = rpool.tile([128, CH], f32, name="r_scr", tag="r_scr")
                r_all = rpool.tile([128, CH], f32, name="r_all", tag="r_all")
                # 1/s as exp(-ln(s)) on the scalar engine (keeps the DVE free;
                # Log and Exp share one activation table set)
                nc.scalar.activation(r_scr[:], r_raw[:], Act.Ln)
                nc.scalar.activation(r_all[:], r_scr[:], Act.Exp, scale=-1.0)
                # evict + normalize O^T chunks (rounds to f32r for projection)
                for h in range(4):
                    nc.vector.tensor_mul(O_allT[32 * h:32 * h + 32, qs],
                                         po[h][0:32, :],
                                         r_all[32 * h:32 * h + 32, :])

            def epilogue_b(q):
                # output projection for this n-chunk
                py = psT.tile([128, 2 * CH], f32, name="py", tag="psT")
                for j in range(CH // 128):
                    ncol = q * CH + j * 128
                    nc.tensor.matmul(py[:, j * C:(j + 1) * C],
                                     lhsT=O_allT[:, ncol:ncol + 128],
                                     rhs=wo_sb[:], start=True, stop=True)
                y_sb = ypool.tile([128, (CH // 128) * C], f32, name="y_sb",
                                  tag="y_sb")
                nc.scalar.copy(y_sb[:], py[:, 0:(CH // 128) * C])
                for j in range(CH // 128):
                    nc.sync.dma_start(
                        y_d[q * CH + j * 128:q * CH + (j + 1) * 128, :],
                        y_sb[:, j * C:(j + 1) * C])

            pending = None
            for q in range(NQ):
                qs = bass.ts(q, CH)
                po = None
                pv_backlog = []
                for t in range(MT):
                    mask_t = maskp.tile([128, CH], f32, tag="mask_t")
                    nc.sync.dma_start(
                        mask_t[:],
                        maskT_d[t * 128:(t + 1) * 128, q * CH:(q + 1) * CH])
                    ti = t % TE
                    if ti == 0:
                        T_tile = tpool.tile([128, TE * 4 * CH], f32,
                                            name="T_tile", tag="T_tile")
                        P_tile = ppool.tile([128, TE * 4 * CH], f32r,
                                            name="P_tile", tag="P_tile")
                    for pair in range(2):
                        pt = psT.tile([128, 2 * CH], f32, name="pt", tag="psT")
                        for hh in range(2):
                            h = 2 * pair + hh
                            nc.tensor.matmul(
                                pt[:, hh * CH:(hh + 1) * CH],
                                lhsT=kT_sb[32 * h:32 * h + 32,
                                           t * 128:(t + 1) * 128],
                                rhs=qT_sb[32 * h:32 * h + 32, qs],
                                start=True, stop=True,
                                tile_position=(32 * h, 0))
                        # fused PSUM-evict + mask multiply (mask repeated 2x)
                        mrep = bass.AP(tensor=mask_t[:].tensor,
                                       offset=mask_t[:].offset,
                                       ap=[mask_t[:].ap[0], [0, 2],
                                           mask_t[:].ap[-1]])
                        dst = T_tile[:, (ti * 4 + pair * 2) * CH:
                                     (ti * 4 + pair * 2 + 2) * CH]
                        unit = (q * MT + t) * 2 + pair
                        if (unit * gp_frac) % 8 < gp_frac:
                            # relieve the DVE: ACT evicts PSUM, GPSIMD
                            # does the elementwise multiply in SBUF
                            nc.scalar.copy(dst, pt[:])
                            nc.gpsimd.tensor_mul(dst, dst, mrep)
                        else:
                            nc.vector.tensor_mul(dst, pt[:], mrep)
                    if ti == TE - 1:
                        nc.scalar.activation(P_tile[:], T_tile[:], Act.Exp)
                        pv_backlog.append((t - (TE - 1), P_tile))
                        if t == TE - 1:
                            # first m-tile pair of this pass is in flight:
                            # drain the previous pass's normalize chain, then
                            # set up this pass's PSUM accumulators.
                            if pending is not None:
                                epilogue_a(*pending)
                            po = [psO.tile([128, CH], f32, name="po",
                                           tag="psO") for _ in range(4)]
                            for h in range(4):
                                nc.tensor.matmul(po[h][:], lhsT=zc[0:1, :],
                                                 rhs=zrow[0:1, :],
                                                 start=True, stop=True)
                        if t == min(3 * TE - 1, MT - 1) and pending is not None:
                            # projection of the previous pass, late enough
                            # that its O_allT inputs are long since ready
                            epilogue_b(pending[0])
                            pending = None
                        for tb0, P_t in pv_backlog:
                            for tj in range(TE):
                                tb = tb0 + tj
                                for h in range(4):
                                    vs = v_store[:, tb * VW + 33 * h:
                                                 tb * VW + 33 * h + 33]
                                    nc.tensor.matmul(
                                        po[h][0:33, :],
                                        lhsT=vs,
                                        rhs=P_t[:, (tj * 4 + h) * CH:
                                                (tj * 4 + h + 1) * CH],
                                        start=False, stop=(tb == MT - 1),
                                        skip_group_check=True)
                        pv_backlog = []
                pending = (q, po)
            epilogue_a(*pending)
            epilogue_b(pending[0])
            pending = None
    nc.finalize()
    return nc


def host_prep(x, interaction_mask, W_qkv, b_qkv, W_out, b_out, n=N):
    """Build per-core input bindings (host-side sharding + layout prep)."""
    x = np.asarray(x, np.float32)
    interaction_mask = np.asarray(interaction_mask, np.float32)
    W_qkv = np.asarray(W_qkv, np.float32)
    b_qkv = np.asarray(b_qkv, np.float32)
    W_out = np.asarray(W_out, np.float32)

    maskT = np.ascontiguousarray(
        ((0.1 + 0.9 * interaction_mask) * SCALE).T).astype(np.float32)
    Wr = W_qkv.reshape(C, 3, H, D)
    br = b_qkv.reshape(3, H, D)
    Wor = W_out.reshape(H, D, C)

    in_maps = []
    for core in range(NCORES):
        b = core // 2
        g = core % 2
        hs = slice(4 * g, 4 * g + 4)
        xT = np.ascontiguousarray(x[b].T)  # [C, n]

        wq = np.concatenate([Wr[:, 0, hs, :].reshape(C, HD),
                             br[0, hs, :].reshape(1, HD)], axis=0)
        wk = np.concatenate([Wr[:, 1, hs, :].reshape(C, HD),
                             br[1, hs, :].reshape(1, HD)], axis=0)
        # v augmented with a ones column per head: weights 0, bias 1
        wv_blocks, bv_blocks = [], []
        for h in range(4 * g, 4 * g + 4):
            wv_blocks.append(np.concatenate(
                [Wr[:, 2, h, :], np.zeros((C, 1), np.float32)], axis=1))
            bv_blocks.append(np.concatenate(
                [br[2, h, :], np.ones((1,), np.float32)]))
        wv = np.concatenate(
            [np.concatenate(wv_blocks, axis=1),
             np.concatenate(bv_blocks)[None, :]], axis=0)  # [C+1, VW]
        wo = np.ascontiguousarray(Wor[hs].reshape(HD, C))

        in_maps.append({
            "xT": xT,
            "maskT": maskT,
            "wq": np.ascontiguousarray(wq),
            "wk": np.ascontiguousarray(wk),
            "wv": np.ascontiguousarray(wv),
            "wo": wo,
        })
    return in_maps


_PROGRAM = {}


def get_program(**kwargs):
    key = tuple(sorted(kwargs.items()))
    if key not in _PROGRAM:
        _PROGRAM[key] = build_program(**kwargs)
    return _PROGRAM[key]


def combine_outputs(results, b_out):
    """results: list of 8 per-core {name: np.ndarray}. Sums head-group
    partials per batch and adds the output bias."""
    b_out = np.asarray(b_out, np.float32)
    out = np.empty((B, N, C), np.float32)
    for b in range(B):
        out[b] = results[2 * b]["y"] + results[2 * b + 1]["y"] + b_out[None, :]
    return out


def kernel(x, interaction_mask, W_qkv, b_qkv, W_out, b_out):
    from concourse.bass_utils import run_bass_kernel_spmd

    in_maps = host_prep(x, interaction_mask, W_qkv, b_qkv, W_out, b_out)
    nc = get_program()
    res = run_bass_kernel_spmd(nc, in_maps, list(range(NCORES)))
    return combine_outputs(res.results, b_out)



# revision 2
# speedup vs baseline: 1.4018x; 1.4018x over previous
"""Trainium2 Bass kernel: BiologicalAttention (mask-modulated MHA).

Computation (per core: one batch b, 4-head group g):
    qkv = x @ W_qkv + b_qkv            [N, 3, 4, D]
    T   = (q k^T) * D**-0.5 * (0.1 + 0.9*mask)   (scores TRANSPOSED: T[m, n])
    P   = exp(T)   (softmax numerator; denominator via ones-column in V)
    O^T = (P^T V)^T / sums             [4*D, N]
    y_partial = O^T.T @ W_out          (host sums 2 head-group partials + bias)

Changes vs the 300us baseline:
  - bf16 operands for phase-1 projections, V, P (exp output), O, W_out
    and the mask (halves DMA + SBUF; end-to-end ~6e-3 rel err vs the
    2e-2 gate; fp8 was tested numerically and fails: 2.3e-2..5e-2).
    qT/kT stay f32r: the QK^T matmuls use tile_position, and 2-byte
    dtypes there take a separate LDWEIGHTS path that hangs the device.
  - PV matmuls deferred and dripped 1-2 m-tiles per tile slot behind the
    exp (which is split in halves), so the PE never queues an exp-gated
    burst ahead of the next QK^T -> the DVE mask-multiply stream (the
    critical engine at ~58% of total) rarely stalls.
  - The previous pass's normalize/projection epilogue is emitted as soon
    as its PV backlog drains, overlapping the next pass's compute.
  - Phase 1 (QKV projections) interleaved into pass 0's tile slots with
    per-deadline emission (saves ~15us of serial startup).
  - 1/sums via a single DVE reciprocal_approx_fast (saves ACT table
    pressure vs Ln+Exp) broadcast by one DMA.
  - Mask prefetched ~6 tiles ahead; startup DMAs ordered so the first
    QK^T chain's dependencies land first.
  - Optional gp_frac mask-multiplies offloaded to GPSIMD via an ACT
    bf16 eviction (GPSIMD cannot read PSUM; walrus rejects it).

HW pitfalls discovered (the sim accepts all of these; the device does
not): (1) two tile_position matmuls back-to-back into the same PSUM
bank hang the device -- one matmul per bank, or >=1 m-tile of
separation, is required; (2) a matmul with start=True resets the whole
bank's OPEN accumulation group, so po tiles are zeroed once by a
rank-1 matmul and PV accumulates with start=False; (3) multi-dim
packed-weight DMA APs ([[HD,128],[128*HD,4],[1,HD]]) deliver garbage;
(4) ACT partition bases must be 32-aligned.
"""

import numpy as np
from contextlib import ExitStack

import concourse.bass as bass
import concourse.tile as tile
import concourse.mybir as mybir
from concourse import bacc

f32 = mybir.dt.float32
f32r = mybir.dt.float32r
bf16 = mybir.dt.bfloat16
Act = mybir.ActivationFunctionType

B, N, C, H = 4, 2048, 256, 8
D = 32
SCALE = D ** -0.5
HPC = 4                # heads per core
HD = HPC * D           # 128
VW = HPC * (D + 1)     # 132: per-m-tile v block ([v_h | 1] x 4 heads)
NCORES = 8


def build_program(n=N, debug=False, reps=1, gp_frac=0, nq=4, te=2,
                  gp_pos=1, esplit=2, drain_pos=3, mask_eng='sync', raf=True, stage=9):
    NQ = nq
    CH = n // NQ           # 512
    MT = n // 128          # 16 m-tiles
    TE = te                # m-tiles per exp batch
    PCH = n // 4           # projection chunk width (512)
    assert MT % TE == 0 and CH == PCH

    nc = bacc.Bacc("TRN2", target_bir_lowering=False, debug=debug)

    xT_d = nc.dram_tensor("xT", [C, n], bf16, kind="ExternalInput")
    maskT_d = nc.dram_tensor("maskT", [n, n], bf16, kind="ExternalInput")
    wqk_d = nc.dram_tensor("wqk", [4 * 128, HD], bf16, kind="ExternalInput")
    wqkb_d = nc.dram_tensor("wqkb", [2, HD], bf16, kind="ExternalInput")
    wv_d = nc.dram_tensor("wv", [C + 1, VW], bf16, kind="ExternalInput")
    wo_d = nc.dram_tensor("wo", [HD, C], bf16, kind="ExternalInput")
    y_d = nc.dram_tensor("y", [n, C], f32, kind="ExternalOutput")

    with tile.TileContext(nc) as tc, ExitStack() as ctx:
        const = ctx.enter_context(tc.tile_pool(name="const", bufs=1))
        maskp = ctx.enter_context(tc.tile_pool(name="maskp", bufs=24))
        tpool = ctx.enter_context(tc.tile_pool(name="tpool", bufs=2))
        ppool = ctx.enter_context(tc.tile_pool(name="ppool", bufs=2))
        gpool = ctx.enter_context(tc.tile_pool(name="gpool", bufs=2))
        ypool = ctx.enter_context(tc.tile_pool(name="ypool", bufs=2))
        spool = ctx.enter_context(tc.tile_pool(name="spool", bufs=2))
        psT = ctx.enter_context(tc.tile_pool(name="psT", bufs=2, space="PSUM"))
        psO = ctx.enter_context(tc.tile_pool(name="psO", bufs=4, space="PSUM"))

        # ---------------- constants / inputs ----------------
        # DMA order front-loads the first QK^T chain's deps (x cols 0:512,
        # W_q/W_k) and the first mask tiles; the rest follows.
        xr0 = const.tile([128, n], bf16, tag="xr0")
        xr1 = const.tile([128, n], bf16, tag="xr1")
        nc.sync.dma_start(xr0[:, 0:PCH], xT_d[0:128, 0:PCH])
        nc.sync.dma_start(xr1[:, 0:PCH], xT_d[128:256, 0:PCH])
        ones_r = const.tile([1, n], bf16, tag="ones_r")
        nc.vector.memset(ones_r[:], 1.0)
        zc = const.tile([1, 128], bf16, tag="zc")
        z2c = const.tile([1, CH], bf16, tag="z2c")
        nc.vector.memset(zc[:], 0.0)
        nc.vector.memset(z2c[:], 0.0)

        wqk_sb = const.tile([128, 4 * HD], bf16, tag="wqk_sb")
        wv_sb = const.tile([128, 2 * VW], bf16, tag="wv_sb")
        wqkb = const.tile([1, 2 * HD], bf16, tag="wqkb")
        wvb = const.tile([1, VW], bf16, tag="wvb")
        wo_sb = const.tile([128, C], bf16, tag="wo_sb")
        for ci in range(4):
            nc.sync.dma_start(wqk_sb[:, ci * HD:(ci + 1) * HD],
                              wqk_d[ci * 128:(ci + 1) * 128, :])
        nc.sync.dma_start(wqkb[0:1, 0:HD], wqkb_d[0:1, :])
        nc.sync.dma_start(wqkb[0:1, HD:2 * HD], wqkb_d[1:2, :])
        wq_sb = wqk_sb[:, 0:2 * HD]
        wk_sb = wqk_sb[:, 2 * HD:4 * HD]
        wqb = wqkb[0:1, 0:HD]
        wkb = wqkb[0:1, HD:2 * HD]

        mask_tiles = [None] * MT
        mask_gen = {}

        def fetch_mask(gen, t):
            """gen = (rep, pass): which CH-wide mask column block tile t
            currently holds."""
            if mask_gen.get(t) == gen:
                return
            mask_t = maskp.tile([128, CH], bf16, name="mask_t",
                                tag="mask_t")
            getattr(nc, mask_eng).dma_start(
                mask_t[:],
                maskT_d[t * 128:(t + 1) * 128,
                        gen[1] * CH:(gen[1] + 1) * CH])
            mask_tiles[t] = mask_t
            mask_gen[t] = gen

        for t in range(min(6, MT)):
            fetch_mask((0, 0), t)

        nc.sync.dma_start(xr0[:, PCH:n], xT_d[0:128, PCH:n])
        nc.sync.dma_start(xr1[:, PCH:n], xT_d[128:256, PCH:n])
        nc.sync.dma_start(wv_sb[:, 0:VW], wv_d[0:128, :])
        nc.sync.dma_start(wv_sb[:, VW:2 * VW], wv_d[128:256, :])
        nc.sync.dma_start(wvb[:], wv_d[256:257, :])
        nc.sync.dma_start(wo_sb[:], wo_d[:])

        sraw = const.tile([128, CH], f32, tag="sraw")
        r4 = const.tile([128, CH], f32, tag="r4")
        nc.vector.memset(sraw[:], 1.0)
        qT_sb = const.tile([128, n], f32r, tag="qT_sb")
        kT_sb = const.tile([128, n], f32r, tag="kT_sb")
        v_store = const.tile([128, MT * VW], bf16, tag="v_store")
        O_allT = const.tile([128, n], bf16, tag="O_allT")

        for _rep in range(reps):
            # ---------- phase 1 emission helpers (interleaved) ----------
            def emit_qk_chunk(chunk, dst, w_sb, w_b):
                cs = bass.ts(chunk, PCH)
                pq = psT.tile([128, PCH], f32, name="pq", tag="psT")
                nc.tensor.matmul(pq[:], lhsT=w_sb[:, 0:HD], rhs=xr0[:, cs],
                                 start=True, stop=False)
                nc.tensor.matmul(pq[:], lhsT=w_sb[:, HD:2 * HD],
                                 rhs=xr1[:, cs], start=False, stop=False)
                nc.tensor.matmul(pq[:], lhsT=w_b, rhs=ones_r[0:1, cs],
                                 start=False, stop=True)
                nc.scalar.copy(dst[:, cs], pq[:])

            def emit_v_tile(t, evict_eng):
                ms = bass.ts(t, 128)
                pv = psT.tile([128, VW], f32, name="pv", tag="psT")
                nc.tensor.matmul(pv[:], lhsT=xr0[:, ms], rhs=wv_sb[:, 0:VW],
                                 start=True, stop=False)
                nc.tensor.matmul(pv[:], lhsT=xr1[:, ms],
                                 rhs=wv_sb[:, VW:2 * VW],
                                 start=False, stop=False)
                nc.tensor.matmul(pv[:], lhsT=ones_r[0:1, ms], rhs=wvb[0:1, :],
                                 start=False, stop=True)
                if evict_eng == "act":
                    nc.scalar.copy(v_store[:, t * VW:(t + 1) * VW], pv[:])
                else:
                    nc.vector.tensor_copy(v_store[:, t * VW:(t + 1) * VW],
                                          pv[:])

            # worklist keyed by (pass, tile) deadline slot; runs right
            # after that tile's QK+mul emission.
            phase1_slots = {}

            def add_slot(q, t, fn):
                phase1_slots.setdefault((q, t), []).append(fn)

            # prologue: q/k chunk 0 only (covers passes 0-1 / tiles 0-3)
            emit_qk_chunk(0, qT_sb, wq_sb, wqb)
            emit_qk_chunk(0, kT_sb, wk_sb, wkb)
            # k chunk c first needed at pass 0 tile c*PCH/128
            for ci in range(1, 4):
                add_slot(0, max(0, ci * (PCH // 128) - 3),
                         lambda ci=ci: emit_qk_chunk(ci, kT_sb, wk_sb, wkb))
            # v tile t needed at the deferred PV of its group
            # (emitted at tile 4*(t//4)+5, or pass+1 tile 1 for the last
            # group); alternate the eviction engine to balance ACT/DVE.
            for t in range(MT):
                add_slot(0, t, lambda t=t: emit_v_tile(
                    t, "act" if t % 2 else "dve"))
            # q chunk c first needed at pass c*PCH/CH; emit one pass early
            for ci in range(1, 4):
                add_slot(max(0, ci * PCH // CH - 1), min(TE + 2, MT - 1),
                         lambda ci=ci: emit_qk_chunk(ci, qT_sb, wq_sb, wqb))

            # -------------------- phase 2: passes --------------------
            def epilogue_a(q, po):
                qs = bass.ts(q, CH)
                # sums rows (po row 32 per head) -> partitions 32h of sraw
                # (ACT partition bases must be 32-aligned; the junk rows in
                # between were memset once so the reciprocal stays finite)
                for h in range(4):
                    nc.scalar.copy(sraw[32 * h:32 * h + 1, :],
                                   po[h][32:33, :])
                if raf:
                    nc.vector.reciprocal_approx_fast(r4[:], sraw[:])
                else:
                    nc.vector.reciprocal(r4[:], sraw[:])
                # broadcast each head's row across its 32 partitions
                r_all = spool.tile([128, CH], f32, name="r_all", tag="r_all")
                src = r4[:]
                bc = bass.AP(tensor=src.tensor, offset=src.offset,
                             ap=[[32 * src.ap[0][0], 4], [0, 32], src.ap[-1]])
                nc.sync.dma_start(r_all[:], bc)
                # normalize + evict to bf16 O^T
                for h in range(4):
                    nc.vector.tensor_mul(
                        O_allT[32 * h:32 * h + 32, qs],
                        po[h][0:32, :],
                        r_all[32 * h:32 * h + 32, :])

            def epilogue_b(q):
                py = psT.tile([128, (CH // 128) * C], f32, name="py",
                              tag="psT")
                for j in range(CH // 128):
                    ncol = q * CH + j * 128
                    nc.tensor.matmul(py[:, j * C:(j + 1) * C],
                                     lhsT=O_allT[:, ncol:ncol + 128],
                                     rhs=wo_sb[:], start=True, stop=True)
                y_sb = ypool.tile([128, (CH // 128) * C], f32, name="y_sb",
                                  tag="y_sb")
                nc.scalar.copy(y_sb[:], py[:, 0:(CH // 128) * C])
                for j in range(CH // 128):
                    nc.sync.dma_start(
                        y_d[q * CH + j * 128:q * CH + (j + 1) * 128, :],
                        y_sb[:, j * C:(j + 1) * C])

            pending = None
            pv_backlog = []
            pass_po = {}
            ep_stage = [None]

            def ensure_po(q):
                if q in pass_po:
                    return
                pass_po[q] = [psO.tile([128, CH], f32, name="po",
                                       tag="psO") for _ in range(4)]
                for po_ in pass_po[q]:
                    nc.tensor.matmul(po_[:], lhsT=zc[0:1, :],
                                     rhs=z2c[0:1, :], start=True, stop=True)

            def drain_pv(item, ntiles=TE):
                """Emit PV matmuls for the next `ntiles` available m-tiles
                of a backlogged group; True when the group is spent."""
                bq, P_tile, tb0, prog, avail, _at = item
                ensure_po(bq)
                po_ = pass_po[bq]
                for tj in range(prog, min(prog + ntiles, avail)):
                    tb = tb0 + tj
                    for h in range(4):
                        vs = v_store[:, tb * VW + 33 * h:
                                     tb * VW + 33 * h + 33]
                        nc.tensor.matmul(
                            po_[h][0:33, :],
                            lhsT=vs,
                            rhs=P_tile[:, (tj * 4 + h) * CH:
                                       (tj * 4 + h + 1) * CH],
                            start=False, stop=(tb == MT - 1),
                            skip_group_check=True)
                item[3] = min(prog + ntiles, avail)
                return item[3] >= TE

            for q in range(NQ):
                qs = bass.ts(q, CH)
                gen = (_rep, q)
                nxt = (_rep, q + 1) if q < NQ - 1 else (_rep + 1, 0)
                for t in range(MT):
                    fetch_mask(gen, t)                # normally a no-op
                    if t + 6 < MT:
                        fetch_mask(gen, t + 6)        # lookahead
                    elif nxt[0] < reps:
                        fetch_mask(nxt, t + 6 - MT)   # next pass prefetch
                    mask_t = mask_tiles[t]
                    ti = t % TE
                    if stage < 2:
                        for fn in phase1_slots.pop((q, t), ()):
                            fn()
                        continue
                    if ti == 0:
                        T_tile = tpool.tile([128, TE * 4 * CH], bf16,
                                            name="T_tile", tag="T_tile")
                    # QK^T per head-pair: 2 matmuls into one [128, 2CH]
                    # psum tile (one full bank per head, v1's proven shape)
                    mrep = bass.AP(tensor=mask_t[:].tensor,
                                   offset=mask_t[:].offset,
                                   ap=[mask_t[:].ap[0], [0, 2],
                                       mask_t[:].ap[-1]])
                    for pair in range(2):
                        pt = psT.tile([128, 2 * CH], f32, name="pt",
                                      tag="psT")
                        for hh in range(2):
                            h = 2 * pair + hh
                            nc.tensor.matmul(
                                pt[:, hh * CH:(hh + 1) * CH],
                                lhsT=kT_sb[32 * h:32 * h + 32,
                                           t * 128:(t + 1) * 128],
                                rhs=qT_sb[32 * h:32 * h + 32, qs],
                                start=True, stop=True,
                                tile_position=(32 * h, 0))
                        # fused PSUM-evict + mask multiply (mask repeated 2x)
                        dst = T_tile[:, (ti * 4 + 2 * pair) * CH:
                                     (ti * 4 + 2 * pair + 2) * CH]
                        unit = t * 2 + pair
                        if unit % (2 * TE) == gp_pos and unit // (
                                2 * TE) < gp_frac:
                            # relieve the DVE: ACT evicts to bf16 SBUF,
                            # GPSIMD multiplies there
                            gsb = gpool.tile([128, 2 * CH], bf16,
                                             name="gsb", tag="gsb")
                            nc.scalar.copy(gsb[:], pt[:])
                            nc.gpsimd.tensor_mul(dst, gsb[:], mrep)
                        else:
                            nc.vector.tensor_mul(dst, pt[:], mrep)
                    for fn in phase1_slots.pop((q, t), ()):
                        fn()
                    if stage < 3:
                        continue
                    abs_t = q * MT + t
                    # previous pass's epilogue once its PV fully drained
                    # (emitted BEFORE any drain that would allocate the
                    # next pass's po tiles from the same psum pool)
                    if stage >= 5 and pending is not None and not any(
                            b[0] == pending[0] for b in pv_backlog):
                        if ep_stage[0] is None:
                            epilogue_a(*pending)
                            ep_stage[0] = abs_t
                        elif abs_t >= ep_stage[0] + 2:
                            epilogue_b(pending[0])
                            pending = None
                            ep_stage[0] = None
                    # deferred PV: drip up to 2 m-tiles of PV matmuls
                    # per tile, becoming eligible drain_pos tiles after the
                    # group's exp was emitted, so the PE never queues an
                    # exp-gated burst ahead of a QK.
                    if stage >= 4 and pv_backlog and (drain_pos == 0
                                       or abs_t >= pv_backlog[0][5]
                                       + drain_pos):
                        if drain_pv(pv_backlog[0],
                                    TE if drain_pos == 0 else 2):
                            pv_backlog.pop(0)
                    if esplit == 2 and ti == TE // 2 - 1:
                        P_half = ppool.tile([128, TE * 4 * CH], bf16,
                                            name="P_half", tag="P_tile")
                        hw_ = TE * 4 * CH // 2
                        nc.scalar.activation(P_half[:, 0:hw_],
                                             T_tile[:, 0:hw_], Act.Exp)
                        if stage >= 4:
                            ensure_po(q)
                            pv_backlog.append([q, P_half, t - (TE // 2 - 1),
                                               0, TE // 2, q * MT + t])
                    if ti == TE - 1:
                        if esplit == 2:
                            P_tile = P_half
                            hw_ = TE * 4 * CH // 2
                            nc.scalar.activation(P_tile[:, hw_:2 * hw_],
                                                 T_tile[:, hw_:2 * hw_],
                                                 Act.Exp)
                            for b_ in pv_backlog:
                                if b_[1] is P_tile:
                                    b_[4] = TE
                        else:
                            P_tile = ppool.tile([128, TE * 4 * CH], bf16,
                                                name="P_tile", tag="P_tile")
                            nc.scalar.activation(P_tile[:], T_tile[:], Act.Exp)
                        if stage >= 4:
                            if esplit != 2:
                                pv_backlog.append([q, P_tile, t - (TE - 1),
                                                   0, TE, q * MT + t])
                if stage >= 5:
                    assert pending is None, (
                        f"epilogue for pass {pending and pending[0]} did "
                        f"not complete before pass {q} ended")
                    pending = (q, pass_po[q])
            while pv_backlog:
                drain_pv(pv_backlog[0])
                pv_backlog.pop(0)
            if stage >= 5:
                epilogue_a(*pending)
                epilogue_b(pending[0])
            pending = None
    nc.finalize()
    return nc


def host_prep(x, interaction_mask, W_qkv, b_qkv, W_out, b_out, n=N):
    """Build per-core input bindings (host-side sharding + layout prep)."""
    import ml_dtypes
    nbf = ml_dtypes.bfloat16
    x = np.asarray(x, np.float32)
    interaction_mask = np.asarray(interaction_mask, np.float32)
    W_qkv = np.asarray(W_qkv, np.float32)
    b_qkv = np.asarray(b_qkv, np.float32)
    W_out = np.asarray(W_out, np.float32)

    maskT = np.ascontiguousarray(
        ((0.1 + 0.9 * interaction_mask) * SCALE).T).astype(nbf)
    Wr = W_qkv.reshape(C, 3, H, D)
    br = b_qkv.reshape(3, H, D)
    Wor = W_out.reshape(H, D, C)

    in_maps = []
    for core in range(NCORES):
        b = core // 2
        g = core % 2
        hs = slice(4 * g, 4 * g + 4)
        xT = np.ascontiguousarray(x[b].T).astype(nbf)

        wq_m = Wr[:, 0, hs, :].reshape(C, HD)
        wk_m = Wr[:, 1, hs, :].reshape(C, HD)
        wqk = np.concatenate([wq_m[0:128], wq_m[128:256],
                              wk_m[0:128], wk_m[128:256]], axis=0)
        wqkb = np.stack([br[0, hs, :].reshape(HD),
                         br[1, hs, :].reshape(HD)], axis=0)
        wv_blocks, bv_blocks = [], []
        for h in range(4 * g, 4 * g + 4):
            wv_blocks.append(np.concatenate(
                [Wr[:, 2, h, :], np.zeros((C, 1), np.float32)], axis=1))
            bv_blocks.append(np.concatenate(
                [br[2, h, :], np.ones((1,), np.float32)]))
        wv = np.concatenate(
            [np.concatenate(wv_blocks, axis=1),
             np.concatenate(bv_blocks)[None, :]], axis=0)  # [C+1, VW]
        wo = np.ascontiguousarray(Wor[hs].reshape(HD, C))

        in_maps.append({
            "xT": xT,
            "maskT": maskT,
            "wqk": np.ascontiguousarray(wqk).astype(nbf),
            "wqkb": np.ascontiguousarray(wqkb).astype(nbf),
            "wv": np.ascontiguousarray(wv).astype(nbf),
            "wo": wo.astype(nbf),
        })
    return in_maps


_PROGRAM = {}


def get_program(**kwargs):
    key = tuple(sorted(kwargs.items()))
    if key not in _PROGRAM:
        _PROGRAM[key] = build_program(**kwargs)
    return _PROGRAM[key]


def combine_outputs(results, b_out):
    b_out = np.asarray(b_out, np.float32)
    out = np.empty((B, N, C), np.float32)
    for b in range(B):
        out[b] = results[2 * b]["y"] + results[2 * b + 1]["y"] + b_out[None, :]
    return out


def kernel(x, interaction_mask, W_qkv, b_qkv, W_out, b_out):
    from concourse.bass_utils import run_bass_kernel_spmd

    in_maps = host_prep(x, interaction_mask, W_qkv, b_qkv, W_out, b_out)
    nc = get_program()
    res = run_bass_kernel_spmd(nc, in_maps, list(range(NCORES)))
    return combine_outputs(res.results, b_out)


# revision 3
# speedup vs baseline: 1.6025x; 1.1432x over previous
"""Trainium2 Bass kernel: BiologicalAttention (mask-modulated MHA).

Computation (per core: one batch b, 4-head group g):
    qkv = x @ W_qkv + b_qkv            [N, 3, 4, D]
    T   = (q k^T) * D**-0.5 * (0.1 + 0.9*mask)   (scores TRANSPOSED: T[m, n])
    P   = exp(T)   (softmax numerator; denominator via ones-column in V)
    O^T = (P^T V)^T / sums             [4*D, N]
    y_partial = O^T.T @ W_out          (host sums 2 head-group partials + bias)

Changes vs the 300us baseline:
  - bf16 operands for phase-1 projections, V, P (exp output), O, W_out
    and the mask (halves DMA + SBUF; end-to-end ~6e-3 rel err vs the
    2e-2 gate; fp8 was tested numerically and fails: 2.3e-2..5e-2).
    qT/kT stay f32r: the QK^T matmuls use tile_position, and 2-byte
    dtypes there take a separate LDWEIGHTS path that hangs the device.
  - PV matmuls deferred and dripped 1-2 m-tiles per tile slot behind the
    exp (which is split in halves), so the PE never queues an exp-gated
    burst ahead of the next QK^T -> the DVE mask-multiply stream (the
    critical engine at ~58% of total) rarely stalls.
  - The previous pass's normalize/projection epilogue is emitted as soon
    as its PV backlog drains, overlapping the next pass's compute.
  - Phase 1 (QKV projections) interleaved into pass 0's tile slots with
    per-deadline emission (saves ~15us of serial startup).
  - 1/sums via a single DVE reciprocal_approx_fast (saves ACT table
    pressure vs Ln+Exp) broadcast by one DMA.
  - Mask prefetched ~6 tiles ahead; startup DMAs ordered so the first
    QK^T chain's dependencies land first.
  - Optional gp_frac mask-multiplies offloaded to GPSIMD via an ACT
    bf16 eviction (GPSIMD cannot read PSUM; walrus rejects it).

HW pitfalls discovered (the sim accepts all of these; the device does
not): (1) two tile_position matmuls back-to-back into the same PSUM
bank hang the device -- one matmul per bank, or >=1 m-tile of
separation, is required; (2) a matmul with start=True resets the whole
bank's OPEN accumulation group, so po tiles are zeroed once by a
rank-1 matmul and PV accumulates with start=False; (3) multi-dim
packed-weight DMA APs ([[HD,128],[128*HD,4],[1,HD]]) deliver garbage;
(4) ACT partition bases must be 32-aligned.
"""

import numpy as np
from contextlib import ExitStack

import concourse.bass as bass
import concourse.tile as tile
import concourse.mybir as mybir
from concourse import bacc

f32 = mybir.dt.float32
f32r = mybir.dt.float32r
bf16 = mybir.dt.bfloat16
Act = mybir.ActivationFunctionType

B, N, C, H = 4, 2048, 256, 8
D = 32
SCALE = D ** -0.5
HPC = 4                # heads per core
HD = HPC * D           # 128
VW = HPC * (D + 1)     # 132: per-m-tile v block ([v_h | 1] x 4 heads)
NCORES = 8


def build_program(n=N, debug=False, reps=1, gp_frac=0, nq=4, te=2,
                  gp_pos=1, esplit=2, drain_pos=4, mask_eng='sync', raf=True, stage=9):
    NQ = nq
    CH = n // NQ           # 512
    MT = n // 128          # 16 m-tiles
    TE = te                # m-tiles per exp batch
    PCH = n // 4           # projection chunk width (512)
    assert MT % TE == 0 and CH == PCH

    nc = bacc.Bacc("TRN2", target_bir_lowering=False, debug=debug)

    xT_d = nc.dram_tensor("xT", [C, n], bf16, kind="ExternalInput")
    maskT_d = nc.dram_tensor("maskT", [n, n], bf16, kind="ExternalInput")
    wqk_d = nc.dram_tensor("wqk", [4 * 128, HD], bf16, kind="ExternalInput")
    wqkb_d = nc.dram_tensor("wqkb", [2, HD], bf16, kind="ExternalInput")
    wv_d = nc.dram_tensor("wv", [C + 1, VW], bf16, kind="ExternalInput")
    wo_d = nc.dram_tensor("wo", [HD, C], bf16, kind="ExternalInput")
    y_d = nc.dram_tensor("y", [n, C], f32, kind="ExternalOutput")

    with tile.TileContext(nc) as tc, ExitStack() as ctx:
        const = ctx.enter_context(tc.tile_pool(name="const", bufs=1))
        maskp = ctx.enter_context(tc.tile_pool(name="maskp", bufs=24))
        tpool = ctx.enter_context(tc.tile_pool(name="tpool", bufs=2))
        ppool = ctx.enter_context(tc.tile_pool(name="ppool", bufs=2))
        gpool = ctx.enter_context(tc.tile_pool(name="gpool", bufs=2))
        ypool = ctx.enter_context(tc.tile_pool(name="ypool", bufs=2))
        spool = ctx.enter_context(tc.tile_pool(name="spool", bufs=2))
        psT = ctx.enter_context(tc.tile_pool(name="psT", bufs=2, space="PSUM"))
        psO = ctx.enter_context(tc.tile_pool(name="psO", bufs=4, space="PSUM"))

        # ---------------- constants / inputs ----------------
        # DMA order front-loads the first QK^T chain's deps (x cols 0:512,
        # W_q/W_k) and the first mask tiles; the rest follows.
        xr0 = const.tile([128, n], bf16, tag="xr0")
        xr1 = const.tile([128, n], bf16, tag="xr1")
        nc.sync.dma_start(xr0[:, 0:PCH], xT_d[0:128, 0:PCH])
        nc.sync.dma_start(xr1[:, 0:PCH], xT_d[128:256, 0:PCH])
        ones_r = const.tile([1, n], bf16, tag="ones_r")
        nc.vector.memset(ones_r[:], 1.0)
        zc = const.tile([1, 128], bf16, tag="zc")
        z2c = const.tile([1, CH], bf16, tag="z2c")
        nc.vector.memset(zc[:], 0.0)
        nc.vector.memset(z2c[:], 0.0)

        wqk_sb = const.tile([128, 4 * HD], bf16, tag="wqk_sb")
        wv_sb = const.tile([128, 2 * VW], bf16, tag="wv_sb")
        wqkb = const.tile([1, 2 * HD], bf16, tag="wqkb")
        wvb = const.tile([1, VW], bf16, tag="wvb")
        wo_sb = const.tile([128, C], bf16, tag="wo_sb")
        for ci in range(4):
            nc.sync.dma_start(wqk_sb[:, ci * HD:(ci + 1) * HD],
                              wqk_d[ci * 128:(ci + 1) * 128, :])
        nc.sync.dma_start(wqkb[0:1, 0:HD], wqkb_d[0:1, :])
        nc.sync.dma_start(wqkb[0:1, HD:2 * HD], wqkb_d[1:2, :])
        wq_sb = wqk_sb[:, 0:2 * HD]
        wk_sb = wqk_sb[:, 2 * HD:4 * HD]
        wqb = wqkb[0:1, 0:HD]
        wkb = wqkb[0:1, HD:2 * HD]

        mask_tiles = [None] * MT
        mask_gen = {}

        def fetch_mask(gen, t):
            """gen = (rep, pass): which CH-wide mask column block tile t
            currently holds."""
            if mask_gen.get(t) == gen:
                return
            mask_t = maskp.tile([128, CH], bf16, name="mask_t",
                                tag="mask_t")
            getattr(nc, mask_eng).dma_start(
                mask_t[:],
                maskT_d[t * 128:(t + 1) * 128,
                        gen[1] * CH:(gen[1] + 1) * CH])
            mask_tiles[t] = mask_t
            mask_gen[t] = gen

        for t in range(min(6, MT)):
            fetch_mask((0, 0), t)

        nc.sync.dma_start(xr0[:, PCH:n], xT_d[0:128, PCH:n])
        nc.sync.dma_start(xr1[:, PCH:n], xT_d[128:256, PCH:n])
        nc.sync.dma_start(wv_sb[:, 0:VW], wv_d[0:128, :])
        nc.sync.dma_start(wv_sb[:, VW:2 * VW], wv_d[128:256, :])
        nc.sync.dma_start(wvb[:], wv_d[256:257, :])
        nc.sync.dma_start(wo_sb[:], wo_d[:])

        sraw = const.tile([128, CH], f32, tag="sraw")
        r4 = const.tile([128, CH], f32, tag="r4")
        nc.vector.memset(sraw[:], 1.0)
        qT_sb = const.tile([128, n], f32r, tag="qT_sb")
        kT_sb = const.tile([128, n], f32r, tag="kT_sb")
        v_store = const.tile([128, MT * VW], bf16, tag="v_store")
        O_allT = const.tile([128, n], bf16, tag="O_allT")

        for _rep in range(reps):
            # ---------- phase 1 emission helpers (interleaved) ----------
            def emit_qk_chunk(chunk, dst, w_sb, w_b):
                cs = bass.ts(chunk, PCH)
                pq = psT.tile([128, PCH], f32, name="pq", tag="psT")
                nc.tensor.matmul(pq[:], lhsT=w_sb[:, 0:HD], rhs=xr0[:, cs],
                                 start=True, stop=False)
                nc.tensor.matmul(pq[:], lhsT=w_sb[:, HD:2 * HD],
                                 rhs=xr1[:, cs], start=False, stop=False)
                nc.tensor.matmul(pq[:], lhsT=w_b, rhs=ones_r[0:1, cs],
                                 start=False, stop=True)
                nc.scalar.copy(dst[:, cs], pq[:])

            def emit_v_tile(t, evict_eng):
                ms = bass.ts(t, 128)
                pv = psT.tile([128, VW], f32, name="pv", tag="psT")
                nc.tensor.matmul(pv[:], lhsT=xr0[:, ms], rhs=wv_sb[:, 0:VW],
                                 start=True, stop=False)
                nc.tensor.matmul(pv[:], lhsT=xr1[:, ms],
                                 rhs=wv_sb[:, VW:2 * VW],
                                 start=False, stop=False)
                nc.tensor.matmul(pv[:], lhsT=ones_r[0:1, ms], rhs=wvb[0:1, :],
                                 start=False, stop=True)
                if evict_eng == "act":
                    nc.scalar.copy(v_store[:, t * VW:(t + 1) * VW], pv[:])
                else:
                    nc.vector.tensor_copy(v_store[:, t * VW:(t + 1) * VW],
                                          pv[:])

            # worklist keyed by (pass, tile) deadline slot; runs right
            # after that tile's QK+mul emission.
            phase1_slots = {}

            def add_slot(q, t, fn):
                phase1_slots.setdefault((q, t), []).append(fn)

            # prologue: q/k chunk 0 only (covers passes 0-1 / tiles 0-3)
            emit_qk_chunk(0, qT_sb, wq_sb, wqb)
            emit_qk_chunk(0, kT_sb, wk_sb, wkb)
            # k chunk c first needed at pass 0 tile c*PCH/128
            for ci in range(1, 4):
                add_slot(0, max(0, ci * (PCH // 128) - 3),
                         lambda ci=ci: emit_qk_chunk(ci, kT_sb, wk_sb, wkb))
            # v tile t needed at the deferred PV of its group
            # (emitted at tile 4*(t//4)+5, or pass+1 tile 1 for the last
            # group); alternate the eviction engine to balance ACT/DVE.
            for t in range(MT):
                add_slot(0, t, lambda t=t: emit_v_tile(
                    t, "act" if t % 2 else "dve"))
            # q chunk c first needed at pass c*PCH/CH; emit one pass early
            for ci in range(1, 4):
                add_slot(max(0, ci * PCH // CH - 1), min(TE + 2, MT - 1),
                         lambda ci=ci: emit_qk_chunk(ci, qT_sb, wq_sb, wqb))

            # -------------------- phase 2: passes --------------------
            def epilogue_a(q, po):
                qs = bass.ts(q, CH)
                # sums rows (po row 32 per head) -> partitions 32h of sraw
                # (ACT partition bases must be 32-aligned; the junk rows in
                # between were memset once so the reciprocal stays finite)
                for h in range(4):
                    nc.scalar.copy(sraw[32 * h:32 * h + 1, :],
                                   po[h][32:33, :])
                if raf:
                    nc.vector.reciprocal_approx_fast(r4[:], sraw[:])
                else:
                    nc.vector.reciprocal(r4[:], sraw[:])
                # broadcast each head's row across its 32 partitions
                r_all = spool.tile([128, CH], f32, name="r_all", tag="r_all")
                src = r4[:]
                bc = bass.AP(tensor=src.tensor, offset=src.offset,
                             ap=[[32 * src.ap[0][0], 4], [0, 32], src.ap[-1]])
                nc.sync.dma_start(r_all[:], bc)
                # normalize + evict to bf16 O^T
                for h in range(4):
                    nc.vector.tensor_mul(
                        O_allT[32 * h:32 * h + 32, qs],
                        po[h][0:32, :],
                        r_all[32 * h:32 * h + 32, :])

            def epilogue_b(q):
                py = psT.tile([128, (CH // 128) * C], f32, name="py",
                              tag="psT")
                for j in range(CH // 128):
                    ncol = q * CH + j * 128
                    nc.tensor.matmul(py[:, j * C:(j + 1) * C],
                                     lhsT=O_allT[:, ncol:ncol + 128],
                                     rhs=wo_sb[:], start=True, stop=True)
                y_sb = ypool.tile([128, (CH // 128) * C], f32, name="y_sb",
                                  tag="y_sb")
                nc.scalar.copy(y_sb[:], py[:, 0:(CH // 128) * C])
                for j in range(CH // 128):
                    nc.sync.dma_start(
                        y_d[q * CH + j * 128:q * CH + (j + 1) * 128, :],
                        y_sb[:, j * C:(j + 1) * C])

            pending = None
            pv_backlog = []
            pass_po = {}
            ep_stage = [None]

            def ensure_po(q):
                if q in pass_po:
                    return
                pass_po[q] = [psO.tile([128, CH], f32, name="po",
                                       tag="psO") for _ in range(4)]
                for po_ in pass_po[q]:
                    nc.tensor.matmul(po_[:], lhsT=zc[0:1, :],
                                     rhs=z2c[0:1, :], start=True, stop=True)

            def drain_pv(item, ntiles=TE):
                """Emit PV matmuls for the next `ntiles` available m-tiles
                of a backlogged group; True when the group is spent."""
                bq, P_tile, tb0, prog, avail, _at = item
                ensure_po(bq)
                po_ = pass_po[bq]
                for tj in range(prog, min(prog + ntiles, avail)):
                    tb = tb0 + tj
                    for h in range(4):
                        vs = v_store[:, tb * VW + 33 * h:
                                     tb * VW + 33 * h + 33]
                        nc.tensor.matmul(
                            po_[h][0:33, :],
                            lhsT=vs,
                            rhs=P_tile[:, (tj * 4 + h) * CH:
                                       (tj * 4 + h + 1) * CH],
                            start=False, stop=(tb == MT - 1),
                            skip_group_check=True)
                item[3] = min(prog + ntiles, avail)
                return item[3] >= TE

            for q in range(NQ):
                qs = bass.ts(q, CH)
                gen = (_rep, q)
                nxt = (_rep, q + 1) if q < NQ - 1 else (_rep + 1, 0)
                for t in range(MT):
                    fetch_mask(gen, t)                # normally a no-op
                    if t + 6 < MT:
                        fetch_mask(gen, t + 6)        # lookahead
                    elif nxt[0] < reps:
                        fetch_mask(nxt, t + 6 - MT)   # next pass prefetch
                    mask_t = mask_tiles[t]
                    ti = t % TE
                    if stage < 2:
                        for fn in phase1_slots.pop((q, t), ()):
                            fn()
                        continue
                    if ti == 0:
                        T_tile = tpool.tile([128, TE * 4 * CH], bf16,
                                            name="T_tile", tag="T_tile")
                    # QK^T per head-pair: 2 matmuls into one [128, 2CH]
                    # psum tile (one full bank per head, v1's proven shape)
                    mrep = bass.AP(tensor=mask_t[:].tensor,
                                   offset=mask_t[:].offset,
                                   ap=[mask_t[:].ap[0], [0, 2],
                                       mask_t[:].ap[-1]])
                    for pair in range(2):
                        pt = psT.tile([128, 2 * CH], f32, name="pt",
                                      tag="psT")
                        for hh in range(2):
                            h = 2 * pair + hh
                            nc.tensor.matmul(
                                pt[:, hh * CH:(hh + 1) * CH],
                                lhsT=kT_sb[32 * h:32 * h + 32,
                                           t * 128:(t + 1) * 128],
                                rhs=qT_sb[32 * h:32 * h + 32, qs],
                                start=True, stop=True,
                                tile_position=(32 * h, 0))
                        # fused PSUM-evict + mask multiply (mask repeated 2x)
                        dst = T_tile[:, (ti * 4 + 2 * pair) * CH:
                                     (ti * 4 + 2 * pair + 2) * CH]
                        unit = t * 2 + pair
                        if unit % (2 * TE) == gp_pos and unit // (
                                2 * TE) < gp_frac:
                            # relieve the DVE: ACT evicts to bf16 SBUF,
                            # GPSIMD multiplies there
                            gsb = gpool.tile([128, 2 * CH], bf16,
                                             name="gsb", tag="gsb")
                            nc.scalar.copy(gsb[:], pt[:])
                            nc.gpsimd.tensor_mul(dst, gsb[:], mrep)
                        else:
                            nc.vector.tensor_mul(dst, pt[:], mrep)
                    for fn in phase1_slots.pop((q, t), ()):
                        fn()
                    if stage < 3:
                        continue
                    abs_t = q * MT + t
                    # previous pass's epilogue once its PV fully drained
                    # (emitted BEFORE any drain that would allocate the
                    # next pass's po tiles from the same psum pool)
                    if stage >= 5 and pending is not None and not any(
                            b[0] == pending[0] for b in pv_backlog):
                        if ep_stage[0] is None:
                            epilogue_a(*pending)
                            ep_stage[0] = abs_t
                        elif abs_t >= ep_stage[0] + 2:
                            epilogue_b(pending[0])
                            pending = None
                            ep_stage[0] = None
                    # deferred PV: drip up to 2 m-tiles of PV matmuls
                    # per tile, becoming eligible drain_pos tiles after the
                    # group's exp was emitted, so the PE never queues an
                    # exp-gated burst ahead of a QK.
                    if stage >= 4 and pv_backlog and (drain_pos == 0
                                       or abs_t >= pv_backlog[0][5]
                                       + drain_pos):
                        if drain_pv(pv_backlog[0],
                                    TE if drain_pos == 0 else 2):
                            pv_backlog.pop(0)
                    if esplit == 2 and ti == TE // 2 - 1:
                        P_half = ppool.tile([128, TE * 4 * CH], bf16,
                                            name="P_half", tag="P_tile")
                        hw_ = TE * 4 * CH // 2
                        nc.scalar.activation(P_half[:, 0:hw_],
                                             T_tile[:, 0:hw_], Act.Exp)
                        if stage >= 4:
                            ensure_po(q)
                            pv_backlog.append([q, P_half, t - (TE // 2 - 1),
                                               0, TE // 2, q * MT + t])
                    if ti == TE - 1:
                        if esplit == 2:
                            P_tile = P_half
                            hw_ = TE * 4 * CH // 2
                            nc.scalar.activation(P_tile[:, hw_:2 * hw_],
                                                 T_tile[:, hw_:2 * hw_],
                                                 Act.Exp)
                            for b_ in pv_backlog:
                                if b_[1] is P_tile:
                                    b_[4] = TE
                        else:
                            P_tile = ppool.tile([128, TE * 4 * CH], bf16,
                                                name="P_tile", tag="P_tile")
                            nc.scalar.activation(P_tile[:], T_tile[:], Act.Exp)
                        if stage >= 4:
                            if esplit != 2:
                                pv_backlog.append([q, P_tile, t - (TE - 1),
                                                   0, TE, q * MT + t])
                if stage >= 5:
                    assert pending is None, (
                        f"epilogue for pass {pending and pending[0]} did "
                        f"not complete before pass {q} ended")
                    pending = (q, pass_po[q])
            while pv_backlog:
                drain_pv(pv_backlog[0])
                pv_backlog.pop(0)
            if stage >= 5:
                epilogue_a(*pending)
                epilogue_b(pending[0])
            pending = None
    nc.finalize()
    return nc


def host_prep(x, interaction_mask, W_qkv, b_qkv, W_out, b_out, n=N):
    """Build per-core input bindings (host-side sharding + layout prep)."""
    import ml_dtypes
    nbf = ml_dtypes.bfloat16
    x = np.asarray(x, np.float32)
    interaction_mask = np.asarray(interaction_mask, np.float32)
    W_qkv = np.asarray(W_qkv, np.float32)
    b_qkv = np.asarray(b_qkv, np.float32)
    W_out = np.asarray(W_out, np.float32)

    maskT = np.ascontiguousarray(
        ((0.1 + 0.9 * interaction_mask) * SCALE).T).astype(nbf)
    Wr = W_qkv.reshape(C, 3, H, D)
    br = b_qkv.reshape(3, H, D)
    Wor = W_out.reshape(H, D, C)

    in_maps = []
    for core in range(NCORES):
        b = core // 2
        g = core % 2
        hs = slice(4 * g, 4 * g + 4)
        xT = np.ascontiguousarray(x[b].T).astype(nbf)

        wq_m = Wr[:, 0, hs, :].reshape(C, HD)
        wk_m = Wr[:, 1, hs, :].reshape(C, HD)
        wqk = np.concatenate([wq_m[0:128], wq_m[128:256],
                              wk_m[0:128], wk_m[128:256]], axis=0)
        wqkb = np.stack([br[0, hs, :].reshape(HD),
                         br[1, hs, :].reshape(HD)], axis=0)
        wv_blocks, bv_blocks = [], []
        for h in range(4 * g, 4 * g + 4):
            wv_blocks.append(np.concatenate(
                [Wr[:, 2, h, :], np.zeros((C, 1), np.float32)], axis=1))
            bv_blocks.append(np.concatenate(
                [br[2, h, :], np.ones((1,), np.float32)]))
        wv = np.concatenate(
            [np.concatenate(wv_blocks, axis=1),
             np.concatenate(bv_blocks)[None, :]], axis=0)  # [C+1, VW]
        wo = np.ascontiguousarray(Wor[hs].reshape(HD, C))

        in_maps.append({
            "xT": xT,
            "maskT": maskT,
            "wqk": np.ascontiguousarray(wqk).astype(nbf),
            "wqkb": np.ascontiguousarray(wqkb).astype(nbf),
            "wv": np.ascontiguousarray(wv).astype(nbf),
            "wo": wo.astype(nbf),
        })
    return in_maps


_PROGRAM = {}


def get_program(**kwargs):
    key = tuple(sorted(kwargs.items()))
    if key not in _PROGRAM:
        _PROGRAM[key] = build_program(**kwargs)
    return _PROGRAM[key]


def combine_outputs(results, b_out):
    b_out = np.asarray(b_out, np.float32)
    out = np.empty((B, N, C), np.float32)
    for b in range(B):
        out[b] = results[2 * b]["y"] + results[2 * b + 1]["y"] + b_out[None, :]
    return out


def kernel(x, interaction_mask, W_qkv, b_qkv, W_out, b_out):
    from concourse.bass_utils import run_bass_kernel_spmd

    in_maps = host_prep(x, interaction_mask, W_qkv, b_qkv, W_out, b_out)
    nc = get_program()
    res = run_bass_kernel_spmd(nc, in_maps, list(range(NCORES)))
    return combine_outputs(res.results, b_out)
